# revision 7
# baseline (speedup 1.0000x reference)
"""FlowNetC correlation kernel for Trainium2 (8 NeuronCores, data-parallel over batch).

Problem: out[b, d, y, x] = (1/256) * sum_c in1[b,c,y,x] * in2pad[b,c,y+dy,x+dx]
  with in2 zero-padded by 20 on each spatial side, d = 21*dyi + dxi,
  dy = 2*dyi - 20, dx = 2*dxi - 20 (21x21 = 441 displacements, stride 2).
Shapes: in1/in2 [16, 256, 48, 64] f32 -> out [16, 441, 48, 64] f32.

Strategy per core (2 images):
  - Parity-split rows/cols (displacements are stride-2, so x couples only to
    same-parity padded cols).  Blocks of M=128 = 8 same-parity rows x 16
    same-parity cols.  PE computes Gram blocks G[m=(ys,x_e), n=(rs,u)] =
    sum_c A[c, y, x] * B[c, rp, xp] streaming a [28 rs x 36 u] padded-in2
    window (N=1008, two 504-wide matmuls per c-chunk, accumulated over 2
    c-chunks of 128).
  - PSUM -> SBUF copy (DVE), bulk DMA to a DRAM scratch (m-major).
  - Band gather: out[dyi, dxi] lives at scratch[m=(ys,x_e), rs=ys+dyi,
    u=x_e+dxi] - a diagonal access, legal as an affine multi-dim AP on the
    DRAM side of a DMA (84B dxi-contiguous runs).  Gathered into SBUF tiles
    partitioned by dyi.
  - DVE free-dim shuffle to x-contiguous order, then final DMA to the
    d-major output with 256B runs.
"""

import numpy as np

_CACHE = {}

# ---- geometry (hardcoded for [16, 256, 48, 64]) ----
N_CORES = 8
B2 = 2            # images per core
CH = 2            # channel chunks
CP = 128          # channels per chunk (partition dim)
H, W = 48, 64
PY, YT, YS = 2, 3, 8     # y parity, y tiles, rows per block
PX, XT, XE = 2, 2, 16    # x parity, x tiles, cols per block
NDI = 21                 # dyi / dxi count
RS, U = 28, 36           # rhs window rows / cols (parity space)
NBLK = PY * YT * PX * XT          # 24 blocks per image
RP, CPD = 88, 104                 # padded in2 rows / cols
NN = RS * U                       # 1008 psum free size
BLKSZ = 128 * NN                  # scratch elems per block = 129024
S3F = XE * (B2 * NBLK) * NDI      # S3 free size = 16*48*21 = 16128
S4F = NBLK // PX // XT * NDI * W  # per-b S4 free = (py*yt=6)*21*64 = 8064


def _build():
    import concourse.bacc as bacc
    import concourse.bass as bass
    import concourse.mybir as mybir
    import concourse.tile as tile

    f32 = mybir.dt.float32
    nc = bacc.Bacc("TRN2", target_bir_lowering=False, debug=False,
                   enable_asserts=False, num_devices=N_CORES)

    in1 = nc.dram_tensor("in1", [B2, CH * CP, H, W], f32, kind="ExternalInput")
    in2 = nc.dram_tensor("in2", [B2, CH * CP, H, W], f32, kind="ExternalInput")
    out = nc.dram_tensor("out", [B2, NDI * NDI, H, W], f32, kind="ExternalOutput")

    with tile.TileContext(nc) as tc:
        with tc.tile_pool(name="scr", bufs=1, space="DRAM") as scr_pool:
            scr = scr_pool.tile([B2 * NBLK, 128, NN], f32)

            # ---------------- phase 1+2: Gram blocks -> scratch ----------
            with (
                tc.tile_pool(name="io", bufs=1) as io_pool,
                tc.tile_pool(name="gram", bufs=3) as gram_pool,
                tc.tile_pool(name="psum", bufs=3, space="PSUM") as psum_pool,
            ):
                asb = io_pool.tile([CP, CH, H, W], f32)        # 24.6KB/part
                bsb = io_pool.tile([CP, CH, RP, CPD], f32)     # 73.2KB/part
                # packed weights: [ch][py][yt][px][xt][m=ys*16+x_e]
                apk = io_pool.tile([CP, CH, PY, YT, PX, XT, 128], f32)
                nc.vector.memset(bsb[:], 0.0)

                for b in range(B2):
                    for ch in range(CH):
                        nc.sync.dma_start(asb[:, ch], in1[b, ch * CP:(ch + 1) * CP])
                        nc.sync.dma_start(bsb[:, ch, 20:20 + H, 20:20 + W],
                                          in2[b, ch * CP:(ch + 1) * CP])
                    # pack weights per block; fold in the exact 1/256 norm
                    # apk free layout: (((ch*PY+py)*YT+yt)*PX+px)*XT+xt -> m
                    for ch in range(CH):
                        for py in range(PY):
                            for px in range(PX):
                                for yt in range(YT):
                                    src = bass.AP(
                                        asb.tensor,
                                        ch * H * W + (yt * 16 + py) * W + px,
                                        [[CH * H * W, CP],  # partitions
                                         [32, XT],          # xt
                                         [2 * W, YS],       # ys
                                         [2, XE]])          # x_e
                                    dst = bass.AP(
                                        apk.tensor,
                                        (((ch * PY + py) * YT + yt) * PX + px)
                                        * XT * 128,
                                        [[CH * PY * YT * PX * XT * 128, CP],
                                         [128, XT],
                                         [16, YS],
                                         [1, XE]])
                                    nc.vector.tensor_scalar_mul(dst, src, 1.0 / 256.0)

                    for py in range(PY):
                        for yt in range(YT):
                            y0 = yt * 16 + py
                            for px in range(PX):
                                for xt in range(XT):
                                    x0 = xt * 32 + px
                                    blk = ((py * YT + yt) * PX + px) * XT + xt
                                    ps = psum_pool.tile([128, 1024], f32)
                                    for ch in range(CH):
                                        lhsT = apk[:, ch, py, yt, px, xt, :]
                                        for h in range(2):
                                            rhs = bsb[:, ch,
                                                      y0 + 28 * h:y0 + 28 * h + 27:2,
                                                      x0:x0 + 71:2]
                                            nc.tensor.matmul(
                                                ps[:, 512 * h:512 * h + 504],
                                                lhsT, rhs,
                                                start=(ch == 0), stop=(ch == CH - 1))
                                    sg = gram_pool.tile([128, NN], f32)
                                    nc.vector.tensor_copy(sg[:, 0:504], ps[:, 0:504])
                                    nc.vector.tensor_copy(sg[:, 504:1008], ps[:, 512:1016])
                                    nc.sync.dma_start(scr[b * NBLK + blk], sg[:])

            # ---------------- phase 3: band gather -> shuffle -> out -----
            with tc.tile_pool(name="ext", bufs=1) as s3_pool, \
                 tc.tile_pool(name="ext4", bufs=2) as s4_pool:
                for ys in range(YS):
                    s3 = s3_pool.tile([NDI, S3F], f32)
                    for xe in range(XE):
                        off0 = (ys * XE + xe) * NN + ys * U + xe
                        src = bass.AP(scr.tensor, off0,
                                      [[U, NDI], [BLKSZ, B2 * NBLK], [1, NDI]])
                        dst = bass.AP(s3.tensor, xe * (B2 * NBLK * NDI),
                                      [[S3F, NDI], [NDI, B2 * NBLK], [1, NDI]])
                        eng = nc.sync if xe % 2 == 0 else nc.scalar
                        eng.dma_start(dst, src)

                    for b in range(B2):
                        s4 = s4_pool.tile([NDI, S4F], f32)
                        for py in range(PY):
                            for yt in range(YT):
                                # shuffle S3[x_e][blk][dxi] -> S4[(py,yt)][dxi][x]
                                for px in range(PX):
                                    src = bass.AP(
                                        s3.tensor,
                                        (b * 24 + (py * YT + yt) * 4 + px * 2) * NDI,
                                        [[S3F, NDI],          # dyi (partitions)
                                         [B2 * NBLK * NDI, XE],  # x_e
                                         [NDI, XT],           # xt
                                         [1, NDI]])           # dxi
                                    dst = bass.AP(
                                        s4.tensor,
                                        (py * YT + yt) * NDI * W + px,
                                        [[S4F, NDI],          # dyi (partitions)
                                         [2, XE],             # x_e -> x stride 2
                                         [32, XT],            # xt -> x stride 32
                                         [W, NDI]])           # dxi
                                    nc.vector.tensor_copy(dst, src)
                        for py in range(PY):
                            for yt in range(YT):
                                y = yt * 16 + 2 * ys + py
                                src = bass.AP(
                                    s4.tensor, (py * YT + yt) * NDI * W,
                                    [[S4F, NDI], [W, NDI], [1, W]])
                                dst = bass.AP(
                                    out, b * 441 * H * W + y * W,
                                    [[NDI * H * W, NDI],  # dyi
                                     [H * W, NDI],        # dxi
                                     [1, W]])             # x
                                eng = nc.sync if (py * YT + yt) % 2 == 0 else nc.scalar
                                eng.dma_start(dst, src)

    nc.compile()
    return nc


def _get_nc():
    if "nc" not in _CACHE:
        _CACHE["nc"] = _build()
    return _CACHE["nc"]


def kernel(input1, input2):
    from concourse.bass_utils import run_bass_kernel_spmd

    input1 = np.ascontiguousarray(np.asarray(input1), dtype=np.float32)
    input2 = np.ascontiguousarray(np.asarray(input2), dtype=np.float32)
    nc = _get_nc()
    in_maps = [
        {"in1": input1[i * B2:(i + 1) * B2], "in2": input2[i * B2:(i + 1) * B2]}
        for i in range(N_CORES)
    ]
    res = run_bass_kernel_spmd(nc, in_maps, list(range(N_CORES)))
    return np.concatenate([res.results[i]["out"] for i in range(N_CORES)], axis=0)
